# revision 3
# baseline (speedup 1.0000x reference)
"""KMeans soft-assignment layer (vq_codebook) for 8x TRN2 NeuronCores — v2.

softmax(-||x-c||^2 / T) over K=512 centroids, T=0.1.

Math: softmax is invariant to the per-row ||x||^2 term, so
logits l = (2*x.c - ||c||^2) / T = x @ (20*c)^T - 10*||c||^2, computed by an
f32r matmul with a ones-row appended to x^T (lhsT=[x^T;1], rhs=[20c^T;-10csq]).

v2 vs baseline:
 - Output is the UNNORMALIZED exp(l - m) in BF16 (half the HBM write
   traffic; well within the 2e-2 gate); the softmax division by the row sum
   happens on the host, which cancels the per-row shift m exactly. This
   removes the row-sum accumulate (ACT), reciprocal and normalize-multiply
   (DVE) from the device entirely.
 - maxg="group": ONE 3-D tensor_reduce per G-tile group ([128, G, 512] ->
   [128, G], negate) amortizes the DVE instruction overhead; PSUM is used
   as 2 G-bank mega tiles.
 - maxg="tile": per-tile reduce (baseline-like shallow pipeline).
 - Out DMAs are batched per group (one strided DMA) to keep the SP
   sequencer (~565ns per DMA issue) off the critical path.
"""
import sys

sys.path.insert(0, "/opt/trn_rl_repo")

from contextlib import ExitStack

import numpy as np

import concourse.bacc as bacc
import concourse.bass as bass
import concourse.mybir as mybir
import concourse.tile as tile
from concourse.bass_utils import run_bass_kernel_spmd

N_CORES = 8
B, S_FULL, D = 8, 32768, 64
K = 512
TEMP = 0.1
P = 128
CD = D + 1

F32 = mybir.dt.float32
F32R = mybir.dt.float32r
BF16 = mybir.dt.bfloat16

_NC_CACHE = {}
BEST = dict(pg=1, G=8, chunk=32, in_dma="sync", out_dma="alt",
            bufs_in=3, bufs_e=6)


def _build_nc(
    repeats=1,
    pg=1,
    G=4,
    chunk=16,
    in_dma="scalar",
    out_dma="sync",
    layout="std",
    bufs_in=3,
    bufs_e=6,
    S=S_FULL,
):
    """pg: PSUM-group size (tiles per PSUM mega tile / 3-D max-reduce);
    G: tiles per output-DMA batch (and e-tile width). pg must divide G."""
    n_tiles = S // P
    nc = bacc.Bacc(
        "TRN2", target_bir_lowering=False, debug=False, num_devices=N_CORES
    )
    out = nc.declare_dram_parameter("out", [S, K], BF16, isOutput=True)
    xt = nc.declare_dram_parameter("xt", [CD, S], F32R, isOutput=False)
    rh = nc.declare_dram_parameter("rh", [CD, K], F32R, isOutput=False)
    in_eng = {"sync": nc.sync, "scalar": nc.scalar,
              "pool": nc.gpsimd}[in_dma]

    with tile.TileContext(nc) as tc, ExitStack() as ctx:
        const_pool = ctx.enter_context(tc.tile_pool(name="const", bufs=1))
        in_pool = ctx.enter_context(tc.tile_pool(name="xin", bufs=bufs_in))
        ps_pool = ctx.enter_context(
            tc.tile_pool(name="ps", bufs=8 // pg, space="PSUM")
        )
        e_pool = ctx.enter_context(tc.tile_pool(name="e", bufs=bufs_e))
        m_pool = ctx.enter_context(tc.tile_pool(name="m", bufs=8))

        rhs = const_pool.tile([CD, K], F32R)
        nc.sync.dma_start(rhs[:], rh[:])

        groups = []
        t0 = 0
        while t0 < n_tiles:
            gsz = min(G, n_tiles - t0)
            groups.append((t0, gsz))
            t0 += gsz

        for _rep in range(repeats):
            chunks = {}
            cur_chunk = [-1]

            def need_chunk(c):
                while cur_chunk[0] < c:
                    cur_chunk[0] += 1
                    cc = cur_chunk[0]
                    cw = min(P * chunk, S - cc * P * chunk)
                    xin = in_pool.tile([CD, P * chunk], F32R, tag="x")
                    in_eng.dma_start(
                        xin[:, :cw],
                        xt[:, cc * P * chunk : cc * P * chunk + cw],
                    )
                    chunks[cc] = xin
                return chunks[c]

            for gi, (t0, gsz) in enumerate(groups):
                e = e_pool.tile([P, G * K], BF16, tag="e")
                if layout == "rows2":
                    # Even/odd token interleave: window w covers tiles
                    # (t0+2w, t0+2w+1) = 256 tokens; tile A gets even
                    # tokens, tile B odd, so partition p of the e tile
                    # holds two CONSECUTIVE output rows -> 2KB DRAM
                    # bursts in the group DMA.
                    assert gsz % 2 == 0
                    for w in range(gsz // 2):
                        t = t0 + 2 * w
                        xin = need_chunk(t // chunk)
                        sl = t % chunk
                        base = sl * P
                        for two in range(2):
                            M = ps_pool.tile([P, pg * K], F32)
                            nc.tensor.matmul(
                                M[:, :K],
                                xin[:, base + two : base + 2 * P : 2],
                                rhs[:],
                                start=True, stop=True,
                            )
                            m4 = m_pool.tile([P, pg], F32, tag="m4")
                            nc.vector.tensor_reduce(
                                m4[:, :1], M[:, :K],
                                axis=mybir.AxisListType.X,
                                op=mybir.AluOpType.max,
                                negate=True,
                            )
                            col = (2 * w + two) * K
                            nc.scalar.activation(
                                e[:, col : col + K],
                                M[:, :K],
                                mybir.ActivationFunctionType.Exp,
                                bias=m4[:, :1],
                                scale=1.0,
                            )
                    dram = out[t0 * P : (t0 + gsz) * P, :].rearrange(
                        "(j p two) k -> p j two k", j=gsz // 2, two=2
                    )
                    sbuf = e[:, : gsz * K].rearrange(
                        "p (j two k) -> p j two k", j=gsz // 2, two=2
                    )
                    oeng = nc.sync if (out_dma != "alt" or gi % 2 == 0) \
                        else nc.gpsimd
                    oeng.dma_start(dram, sbuf)
                    continue
                s0 = 0
                while s0 < gsz:
                    psz = min(pg, gsz - s0)
                    M = ps_pool.tile([P, pg * K], F32)
                    for j in range(psz):
                        t = t0 + s0 + j
                        xin = need_chunk(t // chunk)
                        sl = t % chunk
                        nc.tensor.matmul(
                            M[:, j * K : (j + 1) * K],
                            xin[:, sl * P : (sl + 1) * P],
                            rhs[:],
                            start=True, stop=True,
                        )
                    m4 = m_pool.tile([P, pg], F32, tag="m4")
                    if psz > 1:
                        nc.vector.tensor_reduce(
                            m4[:, :psz],
                            M[:, : psz * K].rearrange(
                                "p (j k) -> p j k", j=psz
                            ),
                            axis=mybir.AxisListType.X,
                            op=mybir.AluOpType.max,
                            negate=True,
                        )
                    else:
                        nc.vector.tensor_reduce(
                            m4[:, :1], M[:, :K],
                            axis=mybir.AxisListType.X,
                            op=mybir.AluOpType.max,
                            negate=True,
                        )
                    for j in range(psz):
                        nc.scalar.activation(
                            e[:, (s0 + j) * K : (s0 + j + 1) * K],
                            M[:, j * K : (j + 1) * K],
                            mybir.ActivationFunctionType.Exp,
                            bias=m4[:, j : j + 1],
                            scale=1.0,
                        )
                    s0 += psz
                dram = out[t0 * P : (t0 + gsz) * P, :].rearrange(
                    "(j p) k -> p j k", j=gsz
                )
                sbuf = e[:, : gsz * K].rearrange("p (j k) -> p j k", j=gsz)
                oeng = nc.sync if (out_dma != "alt" or gi % 2 == 0) \
                    else nc.gpsimd
                oeng.dma_start(dram, sbuf)

    nc.compile()
    return nc


def _prep_inputs(x, centroids, mode=None):
    c64 = centroids.astype(np.float64)
    csq = np.sum(c64**2, axis=1)
    rh64 = np.empty((CD, K), np.float64)
    rh64[0:D] = (2.0 / TEMP) * c64.T
    rh64[D] = -csq / TEMP
    rh = rh64.astype(np.float32)
    S = x.shape[1]
    in_maps = []
    for b in range(x.shape[0]):
        xtb = np.empty((CD, S), np.float32)
        xtb[0:D] = x[b].T
        xtb[D] = 1.0
        in_maps.append({"xt": np.ascontiguousarray(xtb), "rh": rh})
    return in_maps


def kernel(x, centroids):
    x = np.asarray(x)
    centroids = np.asarray(centroids)
    in_maps = _prep_inputs(x, centroids)

    if "nc" not in _NC_CACHE:
        _NC_CACHE["nc"] = _build_nc(1, **BEST)
    nc = _NC_CACHE["nc"]

    res = run_bass_kernel_spmd(nc, in_maps, list(range(N_CORES))).results
    outs = []
    for b in range(N_CORES):
        e = np.asarray(res[b]["out"]).astype(np.float32)
        s = e.sum(axis=1, keepdims=True)
        outs.append(e / s)
    return np.stack(outs, axis=0).reshape(B, S_FULL, K)


if __name__ == "__main__":
    xs = np.random.randn(B, S_FULL, D).astype(np.float32)
    cs = np.random.randn(K, D).astype(np.float32)
    o = kernel(xs, cs)
    print(o.shape, o.dtype, o[0, 0, :4])
